# revision 3
# baseline (speedup 1.0000x reference)
"""Distance-discriminator kernel for 8 Trainium2 cores.

Math (reference): for x [N, D],
    S[d] = sum_j x[j,d];  Q[d] = sum_j x[j,d]^2
    sq[i,d] = Q[d] - 2 x[i,d] S[d] + N x[i,d]^2        (= sum_j (x[j,d]-x[i,d])^2)
    out = log(sqrt(sq) + eps) @ W.T + b

Device formulation: complete the square,
    sq = (sqrt(N) x - S/sqrt(N))^2 + C,   C = Q - S^2/N
so one ACT Square pass (per-partition bias, accum_out gives N*C for free) and
one ACT Ln pass (per-partition bias C) produce logd2 = ln(sq) = 2 log(dist).
The eps and the 0.5 factor fold into the GEMM weights (W/2); eps is
negligible because dist ~ sqrt(2N) >> eps.

Sharding: columns d are split across the 8 cores (512 each), so S, Q are
fully local; each core computes a [64, 4096] partial of out.T and a
ReduceScatter sums partials, leaving rank c with out.T rows 8c:8c+8.
Inputs are transposed on the host so d sits on SBUF partitions: reductions
are free-axis, the GEMM needs no on-device transpose, and all DMA is
contiguous.
"""

import numpy as np

import concourse.bacc as bacc
import concourse.bass as bass
import concourse.tile as tile
from concourse import mybir
from concourse.bass_utils import run_bass_kernel_spmd

N = 4096          # rows
D = 4096          # feature columns
OUT = 64
NCORES = 8
DC = D // NCORES  # 512 columns per core
KCH = DC // 128   # 4 partition-chunks per core
NBLK = N // 512   # 8 moving-dim blocks per GEMM bank
SQRT_N = float(np.sqrt(N))

F32 = mybir.dt.float32
_cache: dict = {}


def _build():
    nc = bacc.Bacc(
        "TRN2",
        target_bir_lowering=False,
        debug=False,
        num_devices=NCORES,
    )
    xT = nc.dram_tensor("xT", [DC, N], F32, kind="ExternalInput").ap()
    wT = nc.dram_tensor("wT", [DC, OUT], F32, kind="ExternalInput").ap()
    bb = nc.dram_tensor("bb", [OUT, 1], F32, kind="ExternalInput").ap()
    out = nc.dram_tensor("out", [OUT // NCORES, N], F32, kind="ExternalOutput").ap()

    with tile.TileContext(nc) as tc:
        with (
            tc.tile_pool(name="wp", bufs=1) as wp,
            tc.tile_pool(name="xp", bufs=2) as xp,
            tc.tile_pool(name="st", bufs=KCH) as st,
            tc.tile_pool(name="up", bufs=KCH) as up,
            tc.tile_pool(name="lp", bufs=2) as lp,
            tc.tile_pool(name="pp", bufs=NBLK, space="PSUM") as pp,
            tc.tile_pool(name="dp", bufs=1, space="DRAM") as dp,
        ):
            w_all = wp.tile([128, KCH * OUT], F32, name="w_all", tag="w_all")
            for k in range(KCH):
                nc.sync.dma_start(
                    w_all[:, k * OUT : (k + 1) * OUT], wT[k * 128 : (k + 1) * 128, :]
                )
            bias_b = wp.tile([OUT, 1], F32, name="bias_b", tag="bias_b")
            nc.sync.dma_start(bias_b[:], bb)

            us, Cs = [], []
            for k in range(KCH):
                x_k = xp.tile([128, N], F32, name=f"x_{k}", tag="x")
                nc.sync.dma_start(x_k[:], xT[k * 128 : (k + 1) * 128, :])
                S_k = st.tile([128, 1], F32, name=f"S_{k}", tag="S")
                nc.vector.reduce_sum(S_k[:], x_k[:], axis=mybir.AxisListType.X)
                bA_k = st.tile([128, 1], F32, name=f"bA_{k}", tag="bA")
                nc.vector.tensor_scalar_mul(bA_k[:], S_k[:], -1.0 / SQRT_N)
                u_k = up.tile([128, N], F32, name=f"u_{k}", tag="u")
                acc_k = st.tile([128, 1], F32, name=f"acc_{k}", tag="acc")
                nc.scalar.activation(
                    u_k[:],
                    x_k[:],
                    mybir.ActivationFunctionType.Square,
                    bias=bA_k[:],
                    scale=SQRT_N,
                    accum_out=acc_k[:],
                )
                C_k = st.tile([128, 1], F32, name=f"C_{k}", tag="C")
                nc.vector.tensor_scalar_mul(C_k[:], acc_k[:], 1.0 / N)
                us.append(u_k)
                Cs.append(C_k)

            psums = [
                pp.tile([OUT, 512], F32, name=f"ps_{j}", tag="ps")
                for j in range(NBLK)
            ]
            for k in range(KCH):
                l_k = lp.tile([128, N], F32, name=f"l_{k}", tag="l")
                nc.scalar.activation(
                    l_k[:],
                    us[k][:],
                    mybir.ActivationFunctionType.Ln,
                    bias=Cs[k][:],
                    scale=1.0,
                )
                for j in range(NBLK):
                    nc.tensor.matmul(
                        psums[j][:],
                        lhsT=w_all[:, k * OUT : (k + 1) * OUT],
                        rhs=l_k[:, j * 512 : (j + 1) * 512],
                        start=(k == 0),
                        stop=(k == KCH - 1),
                    )

            out_sb = wp.tile([OUT, N], F32, name="out_sb", tag="out_sb")
            for j in range(NBLK):
                nc.vector.tensor_scalar_add(
                    out_sb[:, j * 512 : (j + 1) * 512], psums[j][:], bias_b[:]
                )

            cc_in = dp.tile([OUT, N], F32, name="cc_in", tag="cc_in")
            cc_out = dp.tile([OUT // NCORES, N], F32, name="cc_out", tag="cc_out")
            nc.sync.dma_start(cc_in[:], out_sb[:])
            nc.gpsimd.collective_compute(
                "ReduceScatter",
                mybir.AluOpType.add,
                replica_groups=[list(range(NCORES))],
                ins=[cc_in.opt()],
                outs=[cc_out.opt()],
            )
            nc.sync.dma_start(out, cc_out[:])

    nc.compile()
    return nc


def _prep_inputs(data, W, b):
    data = np.ascontiguousarray(np.asarray(data, dtype=np.float32))
    W = np.asarray(W, dtype=np.float32)
    b = np.asarray(b, dtype=np.float32)
    W2T = np.ascontiguousarray(W.T * 0.5)          # [D, OUT]
    b8 = np.ascontiguousarray((b / NCORES).reshape(OUT, 1))
    in_maps = []
    for c in range(NCORES):
        xT_c = np.ascontiguousarray(data[:, c * DC : (c + 1) * DC].T)  # [DC, N]
        wT_c = np.ascontiguousarray(W2T[c * DC : (c + 1) * DC, :])     # [DC, OUT]
        in_maps.append({"xT": xT_c, "wT": wT_c, "bb": b8})
    return in_maps


def _run(inputs, trace=False, **kwargs):
    if "nc" not in _cache:
        _cache["nc"] = _build()
    nc = _cache["nc"]
    in_maps = _prep_inputs(inputs["data"], inputs["W"], inputs["b"])
    res = run_bass_kernel_spmd(
        nc, in_maps, core_ids=list(range(NCORES)), trace=trace, **kwargs
    )
    outT = np.concatenate([res.results[c]["out"] for c in range(NCORES)], axis=0)
    return np.ascontiguousarray(outT.T), res


def kernel(data, W, b):
    out, _ = _run({"data": data, "W": W, "b": b})
    return out


# revision 6
# speedup vs baseline: 1.1458x; 1.1458x over previous
"""Distance-discriminator kernel for 8 Trainium2 cores.

Math (reference): for x [N, D],
    S[d] = sum_j x[j,d];  Q[d] = sum_j x[j,d]^2
    sq[i,d] = Q[d] - 2 x[i,d] S[d] + N x[i,d]^2        (= sum_j (x[j,d]-x[i,d])^2)
    out = log(sqrt(sq) + eps) @ W.T + b

Device formulation: complete the square,
    sq = (sqrt(N) x - S/sqrt(N))^2 + C,   C = Q - S^2/N
so one ACT Square pass (per-partition bias, accum_out gives N*C for free) and
one ACT Ln pass (per-partition bias C) produce logd2 = ln(sq) = 2 log(dist).
The eps and the 0.5 factor fold into the GEMM weights (W/2); eps is
negligible because dist ~ sqrt(2N) >> eps.

Sharding: columns d are split across the 8 cores (512 each), so S, Q are
fully local; each core computes a [64, 4096] partial of out.T and a
ReduceScatter sums partials, leaving rank c with out.T rows 8c:8c+8.
Inputs are transposed on the host so d sits on SBUF partitions: reductions
are free-axis, the GEMM needs no on-device transpose, and all DMA is
contiguous.
"""

import numpy as np

import concourse.bacc as bacc
import concourse.bass as bass
import concourse.tile as tile
from concourse import mybir
from concourse.bass_utils import run_bass_kernel_spmd

N = 4096          # rows
D = 4096          # feature columns
OUT = 64
NCORES = 8
DC = D // NCORES  # 512 columns per core
KCH = DC // 128   # 4 partition-chunks per core
NBLK = N // 512   # 8 moving-dim blocks per GEMM bank
SQRT_N = float(np.sqrt(N))

F32 = mybir.dt.float32
_cache: dict = {}


def _build():
    nc = bacc.Bacc(
        "TRN2",
        target_bir_lowering=False,
        debug=False,
        num_devices=NCORES,
    )
    xT = nc.dram_tensor("xT", [DC, N], F32, kind="ExternalInput").ap()
    wT = nc.dram_tensor("wT", [DC, OUT], F32, kind="ExternalInput").ap()
    bb = nc.dram_tensor("bb", [OUT, 1], F32, kind="ExternalInput").ap()
    out = nc.dram_tensor("out", [OUT // NCORES, N], F32, kind="ExternalOutput").ap()

    F32R = mybir.dt.float32r
    with tile.TileContext(nc) as tc:
        with (
            tc.tile_pool(name="wp", bufs=1) as wp,
            tc.tile_pool(name="xp", bufs=2) as xp,
            tc.tile_pool(name="st", bufs=KCH) as st,
            tc.tile_pool(name="up", bufs=KCH) as up,
            tc.tile_pool(name="lp", bufs=3) as lp,
            tc.tile_pool(name="pp", bufs=NBLK, space="PSUM") as pp,
            tc.tile_pool(name="dp", bufs=1, space="DRAM") as dp,
        ):
            w_all = wp.tile([128, KCH * OUT], F32, name="w_all", tag="w_all")
            for k in range(KCH):
                nc.sync.dma_start(
                    w_all[:, k * OUT : (k + 1) * OUT], wT[k * 128 : (k + 1) * 128, :]
                )
            bias_b = wp.tile([OUT, 1], F32, name="bias_b", tag="bias_b")
            nc.sync.dma_start(bias_b[:], bb)
            w_r = wp.tile([128, KCH * OUT], F32R, name="w_r", tag="w_r")
            nc.vector.tensor_copy(w_r[:], w_all[:])

            us, Cs = [], []
            for k in range(KCH):
                x_k = xp.tile([128, N], F32, name=f"x_{k}", tag="x")
                # split the 2 MiB chunk load for earlier compute start and
                # parallel DMA queues
                for s in range(4):
                    nc.sync.dma_start(
                        x_k[:, s * 1024 : (s + 1) * 1024],
                        xT[k * 128 : (k + 1) * 128, s * 1024 : (s + 1) * 1024],
                    )
                # bn_stats per 512-wide segment -> mean/var per partition
                stats_k = st.tile([128, 8, 6], F32, name=f"stats_{k}", tag="stats")
                for s in range(8):
                    nc.vector.bn_stats(
                        stats_k[:, s, :], x_k[:, s * 512 : (s + 1) * 512]
                    )
                mv_k = st.tile([128, 2], F32, name=f"mv_{k}", tag="mv")
                nc.vector.bn_aggr(mv_k[:], stats_k[:])
                # bias_A = -S/sqrt(N) = -sqrt(N)*mean ;  C = Q - S^2/N = N*var
                bA_k = st.tile([128, 1], F32, name=f"bA_{k}", tag="bA")
                nc.vector.tensor_scalar_mul(bA_k[:], mv_k[:, 0:1], -SQRT_N)
                C_k = st.tile([128, 1], F32, name=f"C_{k}", tag="C")
                nc.vector.tensor_scalar_mul(C_k[:], mv_k[:, 1:2], float(N))
                u_k = up.tile([128, N], F32, name=f"u_{k}", tag="u")
                nc.scalar.activation(
                    u_k[:],
                    x_k[:],
                    mybir.ActivationFunctionType.Square,
                    bias=bA_k[:],
                    scale=SQRT_N,
                )
                us.append(u_k)
                Cs.append(C_k)

            psums = [
                pp.tile([OUT, 512], F32, name=f"ps_{j}", tag="ps")
                for j in range(NBLK)
            ]
            out_sb = wp.tile([OUT, N], F32, name="out_sb", tag="out_sb")
            cc_ins = [
                dp.tile([OUT, N // 2], F32, name=f"cc_in{h}", tag=f"cc_in{h}")
                for h in range(2)
            ]
            cc_outs = [
                dp.tile(
                    [OUT // NCORES, N // 2], F32, name=f"cc_out{h}", tag=f"cc_out{h}"
                )
                for h in range(2)
            ]
            HB = NBLK // 2  # n-blocks per half
            for h in range(2):
                for k in range(KCH):
                    l_k = lp.tile([128, N // 2], F32R, name=f"l_{h}_{k}", tag="l")
                    nc.scalar.activation(
                        l_k[:],
                        us[k][:, h * (N // 2) : (h + 1) * (N // 2)],
                        mybir.ActivationFunctionType.Ln,
                        bias=Cs[k][:],
                        scale=1.0,
                    )
                    for jj in range(HB):
                        j = h * HB + jj
                        nc.tensor.matmul(
                            psums[j][:],
                            lhsT=w_r[:, k * OUT : (k + 1) * OUT],
                            rhs=l_k[:, jj * 512 : (jj + 1) * 512],
                            start=(k == 0),
                            stop=(k == KCH - 1),
                        )
                for jj in range(HB):
                    j = h * HB + jj
                    nc.vector.tensor_scalar_add(
                        out_sb[:, j * 512 : (j + 1) * 512], psums[j][:], bias_b[:]
                    )
                nc.sync.dma_start(
                    cc_ins[h][:], out_sb[:, h * (N // 2) : (h + 1) * (N // 2)]
                )
                nc.gpsimd.collective_compute(
                    "ReduceScatter",
                    mybir.AluOpType.add,
                    replica_groups=[list(range(NCORES))],
                    ins=[cc_ins[h].opt()],
                    outs=[cc_outs[h].opt()],
                )
                nc.sync.dma_start(out[:, h * (N // 2) : (h + 1) * (N // 2)], cc_outs[h][:])

    nc.compile()
    return nc


def _prep_inputs(data, W, b):
    data = np.ascontiguousarray(np.asarray(data, dtype=np.float32))
    W = np.asarray(W, dtype=np.float32)
    b = np.asarray(b, dtype=np.float32)
    W2T = np.ascontiguousarray(W.T * 0.5)          # [D, OUT]
    b8 = np.ascontiguousarray((b / NCORES).reshape(OUT, 1))
    in_maps = []
    for c in range(NCORES):
        xT_c = np.ascontiguousarray(data[:, c * DC : (c + 1) * DC].T)  # [DC, N]
        wT_c = np.ascontiguousarray(W2T[c * DC : (c + 1) * DC, :])     # [DC, OUT]
        in_maps.append({"xT": xT_c, "wT": wT_c, "bb": b8})
    return in_maps


def _run(inputs, trace=False, **kwargs):
    if "nc" not in _cache:
        _cache["nc"] = _build()
    nc = _cache["nc"]
    in_maps = _prep_inputs(inputs["data"], inputs["W"], inputs["b"])
    res = run_bass_kernel_spmd(
        nc, in_maps, core_ids=list(range(NCORES)), trace=trace, **kwargs
    )
    outT = np.concatenate([res.results[c]["out"] for c in range(NCORES)], axis=0)
    return np.ascontiguousarray(outT.T), res


def kernel(data, W, b):
    out, _ = _run({"data": data, "W": W, "b": b})
    return out
